# revision 16
# baseline (speedup 1.0000x reference)
"""Multi-head causal attention with RoPE on 8 Trainium2 NeuronCores.

Sharding: tensor-parallel over heads x data-parallel over batch.
Core c handles batch b = c//4 and heads [4*(c%4), 4*(c%4)+4) (Hl=256 of Hd=1024).
Each core computes q/k/v projections for its head slice (column-split Wq/Wk/Wv),
RoPE, causal softmax attention, and a partial output projection (row-split Wo).
The host sums the 4 partial outputs per batch (the "all-reduce").

Device layouts (per core, S=2048, E=1024, Hl=256, D=64):
  xT   [E, S]    x transposed (host-side) so E rides the partition dim
  qT/kT slabs [128, S] x2: partitions = 2 heads x 64 dims, free = seq
  v    16 tiles [128, 260]: partitions = seq chunk, free = 4 heads x (64 dims + ones col)
  scores computed transposed (keys on partitions), softmax Z via ones-column of v,
  normalization by 1/Z broadcast via a DRAM-roundtrip partition-broadcast DMA.

All phases run as ONE software-pipelined stream: attention for slab 0 starts
as soon as its q/k slabs are roped; the remaining projection groups, v chunks,
and the output projection are paced into the attention stream as PE filler so
the tensor engine (the overall floor) never waits on the exp ACTIVATEs that
pace the attention inner loop. Per-chunk score tiles pack both heads
side by side so full chunks need one exp and one affine_select; the trailing
three diagonal chunks of each block are width-trimmed. Warmup matmuls keep
the PE HAM clock-gate warm during the input-DMA ramp; partial outputs are
written fp16 and summed on the host.
"""
import sys

sys.path.insert(0, "/opt/trn_rl_repo")
import numpy as np  # noqa: E402

N_HEADS = 16
B, S, E, HD = 2, 2048, 1024, 1024
D = HD // N_HEADS  # 64
HPC = 4            # heads per core
HL = HPC * D       # 256
NCORES = 8
ROPE_BASE = 10000.0

_built = None


def _build_nc():
    from collections import deque

    import concourse.bass as bass
    import concourse.tile as tile
    from concourse import bacc, mybir

    F32 = mybir.dt.float32
    F16 = mybir.dt.float16
    Exp = mybir.ActivationFunctionType.Exp
    is_ge = mybir.AluOpType.is_ge
    ts = bass.ts

    nc = bacc.Bacc("TRN2", target_bir_lowering=False, debug=False)
    xT_d = nc.dram_tensor("xT", [E, S], F16, kind="ExternalInput").ap()
    wq_d = nc.dram_tensor("wq", [E, HL], F16, kind="ExternalInput").ap()
    wk_d = nc.dram_tensor("wk", [E, HL], F16, kind="ExternalInput").ap()
    wv_d = nc.dram_tensor("wv", [E, HL], F16, kind="ExternalInput").ap()
    wo_d = nc.dram_tensor("wo", [HL, E], F16, kind="ExternalInput").ap()
    cos_d = nc.dram_tensor("cosx", [128, S], F16, kind="ExternalInput").ap()
    sin_d = nc.dram_tensor("sinx", [128, S], F16, kind="ExternalInput").ap()
    out_d = nc.dram_tensor("out", [S, E], F16, kind="ExternalOutput").ap()
    zscr_d = nc.dram_tensor("zscr", [HPC, S], F32).ap()  # internal scratch

    ECH = E // 128   # 8 e-chunks
    SCH = S // 128   # 16 seq chunks
    SB = S // 512    # 4 seq blocks
    swap_mask = []
    for i in range(16):
        swap_mask += [2 * i + 1, 2 * i]

    with tile.TileContext(nc) as tc:
        with (
            tc.tile_pool(name="persist", bufs=1) as pp,
            tc.tile_pool(name="evict", bufs=6) as ev,
            tc.tile_pool(name="bx", bufs=1) as bx,
            tc.tile_pool(name="bswp", bufs=2) as bswp,
            tc.tile_pool(name="cexp", bufs=6) as cexp,
            tc.tile_pool(name="cz", bufs=4) as cz,
            tc.tile_pool(name="crb", bufs=3) as crb,
            tc.tile_pool(name="bps", bufs=2, space="PSUM") as bps,
            tc.tile_pool(name="csc", bufs=2, space="PSUM") as csc,
            tc.tile_pool(name="cpv", bufs=1, space="PSUM") as cpv,
        ):
            # persistent tiles
            qT = [pp.tile([128, S], F16, tag=f"qT{c}", name=f"qT{c}") for c in range(2)]
            kT = [pp.tile([128, S], F16, tag=f"kT{c}", name=f"kT{c}") for c in range(2)]
            vt = [pp.tile([128, HPC * (D + 1)], F16, tag=f"v{t}", name=f"v{t}")
                  for t in range(SCH)]
            oT = [pp.tile([128, S], F16, tag=f"oT{c}", name=f"oT{c}") for c in range(2)]
            cosx = pp.tile([128, S], F16, tag="cosx", name="cosx")
            sinx = pp.tile([128, S], F16, tag="sinx", name="sinx")
            wo_t = pp.tile([128, 2, E], F16, tag="wo", name="wo")

            # PE warmup during the input-DMA ramp: junk matmuls keep the
            # HAM activity monitor busy so real matmuls start at 2.4 GHz
            junk = bx.tile([128, 512], F16, tag="junk", name="junk")
            nc.gpsimd.memset(junk[:], 0.0)

            def vones(t):
                nc.gpsimd.memset(
                    vt[t].rearrange("p (h c) -> p h c", c=D + 1)[:, :, D:D + 1],
                    1.0,
                )

            for _ in range(14):
                wps = bps.tile([128, 512], F32, tag="mm", name="warm")
                nc.tensor.matmul(wps[:], junk[:, 0:128], junk[:],
                                 start=True, stop=True)

            # input DMAs over the three DMA-capable queues
            wq_t = bx.tile([128, ECH, HL], F16, tag="wq", name="wq")
            wk_t = bx.tile([128, ECH, HL], F16, tag="wk", name="wk")
            wv_t = bx.tile([128, ECH, HL], F16, tag="wv", name="wv")

            def wdma(eng, w_t_, w_d_):
                eng.dma_start(
                    out=w_t_[:],
                    in_=w_d_.rearrange("(c p) m -> p c m", p=128),
                )

            xt = [bx.tile([128, S], F16, tag=f"x{e}", name=f"x{e}")
                  for e in range(ECH)]

            def xdma(eng, e, j):
                eng.dma_start(
                    out=xt[e][:, ts(j, 512)],
                    in_=xT_d[e * 128:(e + 1) * 128, ts(j, 512)],
                )

            wdma(nc.sync, wq_t, wq_d)
            wdma(nc.scalar, wk_t, wk_d)
            for j in range(SB):
                for e in range(0, ECH, 2):
                    xdma(nc.sync, e, j)
                for e in range(1, ECH, 2):
                    xdma(nc.gpsimd if j < 2 else nc.scalar, e, j)
                if j == 0:
                    for t in range(4):
                        vones(t)
            for t in range(4, SCH):
                vones(t)
            nc.scalar.dma_start(out=cosx[:], in_=cos_d)
            nc.scalar.dma_start(out=sinx[:], in_=sin_d)
            wdma(nc.scalar, wv_t, wv_d)
            nc.scalar.dma_start(
                out=wo_t[:],
                in_=wo_d.rearrange("(c p) e -> p c e", p=128),
            )

            # ---------------- building blocks ----------------
            def proj_group(w_t_, dest, m, j):
                ps = bps.tile([128, 512], F32, tag="mm", name="mm")
                for e in range(ECH):
                    nc.tensor.matmul(
                        ps[:],
                        w_t_[:, e, m * 128:(m + 1) * 128],
                        xt[e][:, ts(j, 512)],
                        start=(e == 0),
                        stop=(e == ECH - 1),
                    )
                nc.vector.tensor_copy(out=dest[m][:, ts(j, 512)], in_=ps[:])

            def rope(dest, c):
                sw = bswp.tile([128, S], F16, tag="swp", name="swp")
                nc.vector.stream_shuffle(
                    out=sw[:], in_=dest[c][:], mask=swap_mask
                )
                nc.vector.tensor_mul(out=sw[:], in0=sw[:], in1=sinx[:])
                nc.vector.tensor_mul(out=dest[c][:], in0=dest[c][:], in1=cosx[:])
                nc.vector.tensor_add(out=dest[c][:], in0=dest[c][:], in1=sw[:])

            def vproj(t):
                ps = bps.tile([128, 512], F32, tag="mm", name="mmv")
                for e in range(ECH):
                    nc.tensor.matmul(
                        ps[:, 0:HL],
                        xt[e][:, ts(t, 128)],
                        wv_t[:, e, :],
                        start=(e == 0),
                        stop=(e == ECH - 1),
                    )
                nc.vector.tensor_copy(
                    out=vt[t].rearrange("p (h c) -> p h c", c=D + 1)[:, :, 0:D],
                    in_=ps[:, 0:HL].rearrange("p (h c) -> p h c", c=D),
                )

            ndg = [0]

            def d_group(t):
                # output projection + fp16 store for seq chunk t
                for n in range(2):
                    ps = bps.tile([128, 512], F32, tag="mm", name="wops")
                    for c in range(2):
                        nc.tensor.matmul(
                            ps[:],
                            oT[c][:, ts(t, 128)],
                            wo_t[:, c, ts(n, 512)],
                            start=(c == 0),
                            stop=(c == 1),
                        )
                    ot = ev.tile([128, 512], F16, tag="out", name="oev")
                    nc.vector.tensor_copy(out=ot[:], in_=ps[:])
                    eng = nc.sync if ndg[0] % 2 == 0 else nc.gpsimd
                    ndg[0] += 1
                    eng.dma_start(out=out_d[ts(t, 128), ts(n, 512)], in_=ot[:])

            # ---------------- attention ----------------
            def qksv(c):
                hs = [2 * c, 2 * c + 1]
                qs = [qT[c][0:64, :], qT[c][64:128, :]]
                ks = [kT[c][0:64, :], kT[c][64:128, :]]
                vs = [
                    [vt[t].rearrange("p (h c) -> p h c", c=D + 1)[:, h, :]
                     for t in range(SCH)]
                    for h in hs
                ]
                return hs, qs, ks, vs

            def trim_off(t, nt):
                # trailing diagonal chunks are mostly masked: width-trim
                if t == nt - 3:
                    return 128
                if t == nt - 2:
                    return 256
                if t == nt - 1:
                    return 384
                return 0

            sc_of = {}
            pv_of = {}

            def emit_sc(u):
                c, j, t, nt = u
                _, qs, ks, _ = qksv(c)
                off = trim_off(t, nt)
                w = 512 - off
                packed = False  # head-packing is PSUM-bank-fatal: paired MMs drain same bank
                sc = csc.tile([128, 1024], F32, tag="sc", name="sc")
                # head 0 on PE rows 0-63, head 1 on rows 64-127: the two
                # matmuls overlap in the array
                for i in range(2):
                    col = i * w if packed else i * 512 + off
                    nc.tensor.matmul(
                        sc[:, col:col + w],
                        ks[i][:, ts(t, 128)],
                        qs[i][:, j * 512 + off:(j + 1) * 512],
                        start=True,
                        stop=True,
                    )
                sc_of[u] = sc

            def emit_consume(u):
                c, j, t, nt = u
                hs, _, _, vs = qksv(c)
                if t == 0:
                    pv_of[(c, j)] = [
                        cpv.tile([65, 512], F32, tag=f"pv{i}", name=f"pv{i}")
                        for i in range(2)
                    ]
                pv = pv_of[(c, j)]
                sc = sc_of.pop(u)
                off = trim_off(t, nt)
                w = 512 - off
                packed = False
                exm = cexp.tile([128, 1024], F16, tag="ex", name="ex")
                if packed:
                    nc.scalar.activation(
                        out=exm[:, 0:2 * w], in_=sc[:, 0:2 * w],
                        func=Exp, scale=0.125,
                    )
                    ex3 = exm[:, 0:2 * w].rearrange("p (h q) -> p h q", h=2)
                elif off:
                    nc.scalar.activation(
                        out=exm.rearrange("p (h q) -> p h q", h=2)[:, :, off:512],
                        in_=sc.rearrange("p (h q) -> p h q", h=2)[:, :, off:512],
                        func=Exp, scale=0.125,
                    )
                    ex3 = exm.rearrange("p (h q) -> p h q", h=2)[:, :, off:512]
                else:
                    nc.scalar.activation(
                        out=exm[:], in_=sc[:], func=Exp, scale=0.125
                    )
                    ex3 = exm.rearrange("p (h q) -> p h q", h=2)
                if t >= nt - 4:  # diagonal chunk: causal mask, both heads
                    nc.gpsimd.affine_select(
                        out=ex3,
                        in_=ex3,
                        compare_op=is_ge,
                        fill=0.0,
                        base=(j * 512 - t * 128) + off,
                        channel_multiplier=-1,
                        pattern=[[0, 2], [1, w]],
                    )
                for i in range(2):
                    col = i * (w if packed else 512) + (0 if packed else off)
                    nc.tensor.matmul(
                        pv[i][:, off:512],
                        vs[i][t],
                        exm[:, col:col + w],
                        start=(t == 0),
                        stop=(t == nt - 1),
                    )
                if t == nt - 1:
                    # end of block: evict + per-block softmax normalization
                    # (Z -> DRAM -> partition-broadcast, invert, scale)
                    zq = cz.tile([65, 2, 512], F32, tag="zq", name="zq")
                    for i in range(2):
                        nc.vector.tensor_copy(
                            out=oT[c][i * 64:(i + 1) * 64, ts(j, 512)],
                            in_=pv[i][0:64, :],
                        )
                        nc.vector.tensor_copy(
                            out=zq[64:65, i, :], in_=pv[i][64:65, :]
                        )
                    for i in range(2):
                        nc.sync.dma_start(
                            out=zscr_d[hs[i], ts(j, 512)],
                            in_=zq[64:65, i, :],
                        )
                    rb = crb.tile([128, 512], F32, tag="rb", name="rb")
                    for i in range(2):
                        nc.sync.dma_start(
                            out=rb[i * 64:(i + 1) * 64, :],
                            in_=zscr_d[hs[i]:hs[i] + 1, ts(j, 512)]
                            .to_broadcast((64, 512)),
                        )
                    rbr = crb.tile([128, 512], F32, tag="rbr", name="rbr")
                    nc.vector.reciprocal_approx_fast(out=rbr[:], in_=rb[:])
                    nc.vector.tensor_mul(
                        out=oT[c][:, ts(j, 512)],
                        in0=oT[c][:, ts(j, 512)],
                        in1=rbr[:],
                    )

            # ---------------- unified emission ----------------
            # head start: slab-0 projections + rope, v chunks 0-3
            for j in range(SB):
                proj_group(wk_t, kT, 0, j)
            for j in range(SB):
                proj_group(wq_t, qT, 0, j)
            rope(kT, 0)
            rope(qT, 0)
            for t in range(4):
                vproj(t)

            # attention unit order: slab 1's inputs are produced by fillers
            # paced into slab 0's first twelve chunks
            units = []
            for c, j in [(0, 0), (0, 1), (1, 0), (1, 1),
                         (0, 2), (1, 2), (0, 3), (1, 3)]:
                nt = 4 * (j + 1)
                for t in range(nt):
                    units.append((c, j, t, nt))

            # fixed-position fillers (order is semantic: rope of slab 1 must
            # be emitted before any slab-1 score matmul)
            fixed = {
                1: [lambda: proj_group(wk_t, kT, 1, 0), lambda: vproj(4)],
                2: [lambda: proj_group(wk_t, kT, 1, 1), lambda: vproj(5)],
                3: [lambda: proj_group(wk_t, kT, 1, 2), lambda: vproj(6)],
                4: [lambda: proj_group(wk_t, kT, 1, 3), lambda: vproj(7)],
                5: [lambda: proj_group(wq_t, qT, 1, 0)],
                6: [lambda: proj_group(wq_t, qT, 1, 1)],
                7: [lambda: proj_group(wq_t, qT, 1, 2)],
                8: [lambda: proj_group(wq_t, qT, 1, 3)],
                9: [lambda: rope(kT, 1)],
                10: [lambda: rope(qT, 1)],
                11: [lambda: vproj(8)],
            }
            fillers = deque(
                [(lambda t=t: vproj(t)) for t in range(9, SCH)]
            )

            emit_sc(units[0])
            for un in range(1, len(units)):
                emit_sc(units[un])
                emit_consume(units[un - 1])
                cu = units[un - 1]
                for f in fixed.get(un, []):
                    f()
                if un not in fixed and fillers and un % 2 == 0:
                    fillers.popleft()()
                if cu[2] == cu[3] - 1 and cu[0] == 1:
                    # both slabs of block j done: output projection ready
                    for t in range(4 * cu[1], 4 * cu[1] + 4):
                        fillers.append(lambda t=t: d_group(t))
            emit_consume(units[-1])
            for t in range(12, SCH):
                fillers.append(lambda t=t: d_group(t))
            while fillers:
                fillers.popleft()()

    nc.compile()
    return nc


def _rope_tables():
    iexp = np.arange(0, D, 2, dtype=np.float32) / np.float32(D)
    inv_freq = np.reciprocal(np.power(np.float32(ROPE_BASE), iexp))  # (32,) f32
    ang = np.arange(S, dtype=np.float32)[:, None] * inv_freq[None, :]  # (S, 32)
    cos = np.cos(ang).astype(np.float32)  # (S, 32)
    sin = np.sin(ang).astype(np.float32)
    cosx = np.empty((64, S), dtype=np.float32)
    sinx = np.empty((64, S), dtype=np.float32)
    cosx[0::2] = cos.T
    cosx[1::2] = cos.T
    sinx[0::2] = -sin.T
    sinx[1::2] = sin.T
    return (np.tile(cosx, (2, 1)).astype(np.float16),
            np.tile(sinx, (2, 1)).astype(np.float16))  # (128, S) each


def get_nc():
    global _built
    if _built is None:
        _built = _build_nc()
    return _built


def make_in_maps(x, Wq, Wk, Wv, Wo):
    cosx, sinx = _rope_tables()
    in_maps = []
    for c in range(NCORES):
        b, g = c // 4, c % 4
        sl = slice(g * HL, (g + 1) * HL)
        in_maps.append({
            "xT": np.ascontiguousarray(x[b].T).astype(np.float16),
            "wq": np.ascontiguousarray(Wq[:, sl]).astype(np.float16),
            "wk": np.ascontiguousarray(Wk[:, sl]).astype(np.float16),
            "wv": np.ascontiguousarray(Wv[:, sl]).astype(np.float16),
            "wo": np.ascontiguousarray(Wo[sl, :]).astype(np.float16),
            "cosx": cosx,
            "sinx": sinx,
        })
    return in_maps


def gather(results):
    out = np.empty((B, S, E), dtype=np.float32)
    for b in range(B):
        acc = results[4 * b]["out"].astype(np.float32)
        for g in range(1, 4):
            acc += results[4 * b + g]["out"].astype(np.float32)
        out[b] = acc
    return out


def kernel(x, Wq, Wk, Wv, Wo):
    from concourse.bass_utils import run_bass_kernel_spmd

    nc = get_nc()
    in_maps = make_in_maps(
        np.asarray(x), np.asarray(Wq), np.asarray(Wk), np.asarray(Wv), np.asarray(Wo)
    )
    res = run_bass_kernel_spmd(nc, in_maps, list(range(NCORES)))
    return gather(res.results)
